# revision 27
# baseline (speedup 1.0000x reference)
"""Trainium2 Bass kernel for a GAT-style GNN layer (8 NeuronCores, SPMD).

Reference computation:
    h = x @ W                                  # [N, FOUT]
    e = leakyrelu(Wh1[row] + Wh2[col])         # per-edge scores
    att = softmax(e, axis=1)                   # axis of size 1 -> exactly 1.0
    out = elu(segment_sum(att * h[col], row))  # [N, FOUT]

Because the softmax is over a size-1 axis, att == 1.0 exactly, so
out = elu(segment_sum(h[col], row)) and `a` is unused.  Since matmul
commutes with the (linear) segment sum:

    out = elu(segment_sum(x[col], row) @ W)

which lets one SPMD launch do everything: gather raw x rows per edge,
one-hot-matmul segment-sum into s = [rows, 256], transpose s on the PE,
apply W on-chip, ELU, write the (transposed) output slice.

Strategy (single SPMD launch, sharded by destination-node range):
  Host:  bucket edges by (dest window of 128 nodes, source chunk of 25000
         nodes); within each (window-group, chunk) run, segments are padded
         to the max count over cores with index-0 fills and runs are
         128-padded; blocks that straddle two windows get one one-hot
         column per window.
  Device: dma_gather x rows (512B each) from the fp16 x table on 4 SWDGE
         queues; per 128-slot block build S = (iota == rowloc) and
         accumulate S.T @ msg into PSUM (segment sum in x-space); PE
         transpose of s; W matmul; ELU; write outT[:, window].
  Host:  transpose per-core outT back and concatenate.
"""

from contextlib import ExitStack
from dataclasses import dataclass

import ml_dtypes
import numpy as np

import concourse.bass as bass
import concourse.tile as tile
from concourse import bacc, library_config, mybir
from concourse.bass_utils import run_bass_kernel_spmd

F32 = mybir.dt.float32
F16 = mybir.dt.float16
BF16 = mybir.dt.bfloat16
I16 = mybir.dt.int16

P = 128
SBATCH = 8


@dataclass(frozen=True)
class Config:
    n: int = 100000          # nodes
    fin: int = 256           # input features
    fout: int = 128          # output features
    ncores: int = 8
    nchunk: int = 4          # gather-table chunks (int16 index limit)
    gwin: int = 3            # windows per gather super-group
    mbufs: int = 4           # msg tile pool depth

    @property
    def npc(self):
        return self.n // self.ncores

    @property
    def nwin(self):
        return (self.npc + P - 1) // P

    @property
    def chunk(self):
        return self.n // self.nchunk


CFG = Config()


def _ceil(a, b):
    return -(-a // b)


# --------------------------------------------------------------------------
# Static edge layout (shared across cores -> one SPMD program)
# --------------------------------------------------------------------------

@dataclass
class Layout:
    cfg: Config
    seg16: np.ndarray        # [nwin*nchunk] max-over-core segment counts
    seg_off: np.ndarray      # [nwin*nchunk] global slot offset of segment
    runs: dict               # (gi, ch) -> (start, real_len)
    groups: list             # list of window-index lists
    win_entries: dict        # w -> [(ch, blk_in_run, map_col)]
    entry_w: np.ndarray      # [n_map] window of entry
    entry_slot0: np.ndarray  # [n_map] global slot of entry's block start
    bmax: list               # per-chunk max blocks per run
    total_slots: int
    n_map: int


def build_layout(cfg: Config, row, col):
    npc, nwin, nchunk, chunk = cfg.npc, cfg.nwin, cfg.nchunk, cfg.chunk
    ngrp = nwin * nchunk

    per_core = []
    counts = np.zeros((cfg.ncores, ngrp), np.int64)
    for k in range(cfg.ncores):
        sel = (row >= k * npc) & (row < (k + 1) * npc)
        r = (row[sel] - k * npc).astype(np.int64)
        c_ = col[sel].astype(np.int64)
        w = r // P
        rl = r - w * P
        ch = c_ // chunk
        cl = c_ - ch * chunk
        key = w * nchunk + ch
        counts[k] = np.bincount(key, minlength=ngrp)
        per_core.append((key, cl, rl))

    # exact max-over-core segment sizes: only run starts (128) and gather
    # spans (16 via the 1024-aligned subgather grid) need alignment
    seg16 = counts.max(axis=0)
    groups = [list(range(g, min(g + cfg.gwin, nwin)))
              for g in range(0, nwin, cfg.gwin)]

    seg_off = np.zeros(ngrp, np.int64)
    runs = {}
    cur = 0
    for gi, ws in enumerate(groups):
        for ch in range(nchunk):
            start = cur
            for w in ws:
                seg_off[w * nchunk + ch] = cur
                cur += int(seg16[w * nchunk + ch])
            real = cur - start
            cur = start + _ceil(real, P) * P
            runs[(gi, ch)] = (start, real)
    total = cur

    win_entries = {}
    entry_w, entry_slot0 = [], []
    mcol = 0
    for gi, ws in enumerate(groups):
        for w in ws:
            ents = []
            for ch in range(nchunk):
                g = w * nchunk + ch
                slen = int(seg16[g])
                if slen == 0:
                    continue
                rs, _ = runs[(gi, ch)]
                soff = int(seg_off[g])
                b0 = (soff - rs) // P
                b1 = _ceil(soff - rs + slen, P)
                for b in range(b0, b1):
                    ents.append((ch, b, mcol))
                    entry_w.append(w)
                    entry_slot0.append(rs + b * P)
                    mcol += 1
            win_entries[w] = ents

    bmax = [max(_ceil(runs[(gi, ch)][1], P) for gi in range(len(groups)))
            for ch in range(nchunk)]
    return Layout(cfg, seg16, seg_off, runs, groups, win_entries,
                  np.array(entry_w), np.array(entry_slot0), bmax,
                  int(total), mcol), per_core


def build_streams(layout: Layout, key, cl, rl):
    """Per-core edge streams: wrapped int16 gather indices + per-map-entry
    row-local values."""
    total = layout.total_slots
    ngrp = len(layout.seg16)

    order = np.argsort(key, kind="stable")
    skey = key[order]
    scl = cl[order]
    srl = rl[order]
    cnt = np.bincount(key, minlength=ngrp)
    starts = np.concatenate([[0], np.cumsum(cnt)[:-1]])
    rank = np.arange(len(skey)) - starts[skey]
    slot = layout.seg_off[skey] + rank

    idx_local = np.zeros(total, np.int16)
    slot_w = np.full(total, -1, np.int32)
    slot_rl = np.full(total, -1, np.int32)
    idx_local[slot] = scl.astype(np.int16)
    slot_w[slot] = skey // layout.cfg.nchunk
    slot_rl[slot] = srl

    idx_w = np.ascontiguousarray(
        np.tile(idx_local.reshape(-1, 16).T, (8, 1)))          # [128, total/16]

    pos = layout.entry_slot0[:, None] + np.arange(P)[None, :]   # [n_map, 128]
    rl_mat = np.where(slot_w[pos] == layout.entry_w[:, None],
                      slot_rl[pos], -1).astype(np.float16)
    rl_w = np.ascontiguousarray(rl_mat.T)                       # [128, n_map]
    return idx_w, rl_w


# --------------------------------------------------------------------------
# Single launch: gather x + segment-sum (one-hot matmul) + W + ELU
# --------------------------------------------------------------------------

def build_kernel(cfg: Config, layout: Layout):
    nc = bacc.Bacc("TRN2", target_bir_lowering=False, debug=False,
                   num_devices=cfg.ncores, num_swdge_queues=4)
    fin, fout, npc, nchunk, chunk = (cfg.fin, cfg.fout, cfg.npc, cfg.nchunk,
                                     cfg.chunk)
    xt = nc.dram_tensor("xt", [cfg.n, fin], F16, kind="ExternalInput")
    wt = nc.dram_tensor("wt", [fin, fout], BF16, kind="ExternalInput")
    ident = nc.dram_tensor("ident", [P, P], BF16, kind="ExternalInput")
    iota_in = nc.dram_tensor("iota", [P, SBATCH * P], F16, kind="ExternalInput")
    idxs = nc.dram_tensor("idxs", [P, layout.total_slots // 16], I16,
                          kind="ExternalInput")
    rowloc = nc.dram_tensor("rowloc", [P, layout.n_map], F16,
                            kind="ExternalInput")
    outT = nc.dram_tensor("outT", [P, npc], F32, kind="ExternalOutput")

    with tile.TileContext(nc) as tc, ExitStack() as ctx:
        nc.gpsimd.load_library(library_config.mlp)

        cpool = ctx.enter_context(tc.tile_pool(name="const", bufs=1))
        mpool = ctx.enter_context(tc.tile_pool(name="msg", bufs=cfg.mbufs))
        spool = ctx.enter_context(tc.tile_pool(name="sel", bufs=6))
        pspool = ctx.enter_context(tc.tile_pool(name="pss", bufs=2,
                                                space="PSUM"))
        ptpool = ctx.enter_context(tc.tile_pool(name="pst", bufs=2,
                                                space="PSUM"))
        popool = ctx.enter_context(tc.tile_pool(name="pso", bufs=2,
                                                space="PSUM"))
        epool = ctx.enter_context(tc.tile_pool(name="elu", bufs=3))

        # split the idx-stream load (first piece first in program order) so
        # the first group's gathers don't wait for the full stream to land
        n_groups = len(layout.groups)
        split_slot = (layout.runs[(min(1, n_groups - 1), 0)][0]
                      if n_groups > 1 else layout.total_slots)
        idx_t0 = cpool.tile([P, max(split_slot, 16) // 16], I16)
        nc.sync.dma_start(idx_t0[:], idxs.ap()[:, :split_slot // 16])
        idx_t1 = cpool.tile([P, (layout.total_slots - split_slot) // 16], I16)
        nc.sync.dma_start(idx_t1[:], idxs.ap()[:, split_slot // 16:])

        def idx_slice(lo, hi):
            if hi <= split_slot:
                return idx_t0[:, lo // 16:hi // 16]
            return idx_t1[:, (lo - split_slot) // 16:(hi - split_slot) // 16]

        iota_t = cpool.tile([P, SBATCH * P], F16)
        nc.sync.dma_start(iota_t[:], iota_in.ap()[:, :])
        rl_t = cpool.tile([P, layout.n_map], F16)
        nc.sync.dma_start(rl_t[:], rowloc.ap()[:, :])
        id_t = cpool.tile([P, P], BF16)
        nc.sync.dma_start(id_t[:], ident.ap()[:, :])
        w0_t = cpool.tile([P, fout], BF16)
        nc.sync.dma_start(w0_t[:], wt.ap()[0:P, :])
        w1_t = cpool.tile([P, fout], BF16)
        nc.sync.dma_start(w1_t[:], wt.ap()[P:2 * P, :])

        # SWDGE queue q is serviced by Q7 core pair (2q, 2q+1); balance the
        # descriptor-generation load by virtual finish time (uniform cost:
        # all four pairs measure ~9.27us per 1024-idx gather in isolation).
        qcost = [1.0, 1.0, 1.0, 1.0]
        vtime = [0.0, 0.0, 0.0, 0.0]
        for gi, ws in enumerate(layout.groups):
            mts = {}
            subs = []
            for ch in range(nchunk):
                start, real = layout.runs[(gi, ch)]
                if real == 0:
                    continue
                mts[ch] = mpool.tile([P, layout.bmax[ch], fin], F16,
                                     name=f"mt{ch}", tag=f"msg{ch}")
                # gather the full 128-padded span: pad slots carry index 0,
                # so every consumed slot is DMA-written (no NaN garbage).
                r128 = _ceil(real, P) * P
                for a in range(0, r128, 1024):
                    subs.append((ch, start, a, min(a + 1024, r128)))
            subs.sort(key=lambda t: (t[2], t[0]))
            for ch, start, a, b in subs:
                q = min(range(4), key=lambda i: vtime[i])
                vtime[q] += qcost[q] * (b - a) / 1024.0
                nc.gpsimd.dma_gather(
                    mts[ch][:, a // P:b // P, :],
                    xt.ap()[ch * chunk:(ch + 1) * chunk, :],
                    idx_slice(start + a, start + b),
                    b - a, b - a, fin,
                    single_packet=True, queue_num=q)

            for w in ws:
                entries = layout.win_entries[w]
                nt = min(npc - w * P, P)
                if not entries:
                    ot = epool.tile([P, P], F32, tag="out")
                    nc.vector.memset(ot[:], 0.0)
                    nc.sync.dma_start(outT.ap()[:, w * P:w * P + nt],
                                      ot[:, :nt])
                    continue
                ps_s = pspool.tile([P, fin], F32)
                nmm = len(entries)
                mi = 0
                bi = 0
                while bi < nmm:
                    nb = min(SBATCH, nmm - bi)
                    sel = spool.tile([P, SBATCH * P], F16, tag="sel")
                    mc0 = entries[bi][2]
                    nc.vector.tensor_tensor(
                        sel[:, :nb * P], iota_t[:, :nb * P],
                        rl_t[:, mc0:mc0 + nb].to_broadcast([P, nb, P]),
                        op=mybir.AluOpType.is_equal)
                    for j in range(nb):
                        ch, blk, _ = entries[bi + j]
                        nc.tensor.matmul(ps_s[:], sel[:, j * P:(j + 1) * P],
                                         mts[ch][:, blk, :],
                                         start=(mi == 0), stop=(mi == nmm - 1))
                        mi += 1
                    bi += nb
                # s [rows, 256] -> bf16 -> PE transpose -> sT [256-part, rows]
                # (PSUM evacuations on the Scalar engine; DVE is busy with
                # one-hot builds)
                s_sb = epool.tile([P, fin], BF16, tag="s")
                nc.scalar.activation(s_sb[:], ps_s[:],
                                     mybir.ActivationFunctionType.Copy)
                psT = ptpool.tile([P, fin], BF16)
                nc.tensor.transpose(psT[:, 0:P], s_sb[:, 0:P], id_t[:])
                nc.tensor.transpose(psT[:, P:2 * P], s_sb[:, P:2 * P], id_t[:])
                sT = epool.tile([P, fin], BF16, tag="sT")
                nc.scalar.activation(sT[:], psT[:],
                                     mybir.ActivationFunctionType.Copy)
                ps_o = popool.tile([P, fout], F32)
                nc.tensor.matmul(ps_o[:], w0_t[:], sT[:, 0:P],
                                 start=True, stop=False)
                nc.tensor.matmul(ps_o[:], w1_t[:], sT[:, P:2 * P],
                                 start=False, stop=True)
                # ELU: relu(x) - 1 + exp(min(x, 0))
                tmin = epool.tile([P, fout], F32, tag="tmin")
                texp = epool.tile([P, fout], F32, tag="texp")
                trel = epool.tile([P, fout], F32, tag="trel")
                ot = epool.tile([P, fout], F32, tag="out")
                nc.scalar.activation(tmin[:], ps_o[:],
                                     mybir.ActivationFunctionType.Relu,
                                     scale=-1.0)
                nc.scalar.activation(texp[:], tmin[:],
                                     mybir.ActivationFunctionType.Exp,
                                     scale=-1.0)
                nc.vector.tensor_scalar(trel[:], ps_o[:], 0.0, -1.0,
                                        mybir.AluOpType.max,
                                        mybir.AluOpType.add)
                nc.vector.tensor_add(ot[:], texp[:], trel[:])
                nc.sync.dma_start(outT.ap()[:, w * P:w * P + nt], ot[:, :nt])
    nc.compile()
    return nc


# --------------------------------------------------------------------------
# Host orchestration
# --------------------------------------------------------------------------

_NC_CACHE = {}


def _kernel_nc(cfg: Config, layout: Layout):
    key = (cfg.n, cfg.fin, cfg.fout, cfg.ncores, cfg.gwin, cfg.mbufs,
           tuple(layout.seg16.tolist()))
    if key not in _NC_CACHE:
        _NC_CACHE[key] = build_kernel(cfg, layout)
    return _NC_CACHE[key]


def run(x, edge_index, W, a=None, cfg: Config = CFG, trace=False):
    """Full pipeline; returns (out, info dict with exec times)."""
    x = np.asarray(x, np.float32)
    W = np.asarray(W, np.float32)
    edge_index = np.asarray(edge_index)
    row = edge_index[0].astype(np.int64)
    col = edge_index[1].astype(np.int64)
    npc = cfg.npc
    info = {}

    layout, per_core = build_layout(cfg, row, col)
    nc = _kernel_nc(cfg, layout)

    x16 = x.astype(np.float16)
    wt = W.astype(ml_dtypes.bfloat16)
    ident = np.eye(P, dtype=ml_dtypes.bfloat16)
    iota = np.ascontiguousarray(
        np.broadcast_to(np.tile(np.arange(P, dtype=np.float16), SBATCH),
                        (P, SBATCH * P)))
    ins = []
    for k in range(cfg.ncores):
        idx_w, rl_w = build_streams(layout, *per_core[k])
        ins.append({"xt": x16, "wt": wt, "ident": ident, "iota": iota,
                    "idxs": idx_w, "rowloc": rl_w})
    r = run_bass_kernel_spmd(nc, ins, list(range(cfg.ncores)), trace=trace)
    out = np.concatenate(
        [np.ascontiguousarray(r.results[k]["outT"][:, :npc].T)
         for k in range(cfg.ncores)], axis=0)
    info["p1_ns"] = 0
    info["p2_ns"] = r.exec_time_ns
    info["total_slots"] = layout.total_slots
    info["results"] = (r,)
    return out, info


def kernel(x, edge_index, W, a=None, **_ignored):
    out, _ = run(x, edge_index, W, a)
    return out


# revision 28
# speedup vs baseline: 1.0063x; 1.0063x over previous
"""Trainium2 Bass kernel for a GAT-style GNN layer (8 NeuronCores, SPMD).

Reference computation:
    h = x @ W                                  # [N, FOUT]
    e = leakyrelu(Wh1[row] + Wh2[col])         # per-edge scores
    att = softmax(e, axis=1)                   # axis of size 1 -> exactly 1.0
    out = elu(segment_sum(att * h[col], row))  # [N, FOUT]

Because the softmax is over a size-1 axis, att == 1.0 exactly, so
out = elu(segment_sum(h[col], row)) and `a` is unused.  Since matmul
commutes with the (linear) segment sum:

    out = elu(segment_sum(x[col], row) @ W)

which lets one SPMD launch do everything: gather raw x rows per edge,
one-hot-matmul segment-sum into s = [rows, 256], transpose s on the PE,
apply W on-chip, ELU, write the (transposed) output slice.

Strategy (single SPMD launch, sharded by destination-node range):
  Host:  bucket edges by (dest window of 128 nodes, source chunk of 25000
         nodes); within each (window-group, chunk) run, segments are padded
         to the max count over cores with index-0 fills and runs are
         128-padded; blocks that straddle two windows get one one-hot
         column per window.
  Device: dma_gather x rows (512B each) from the fp16 x table on 4 SWDGE
         queues; per 128-slot block build S = (iota == rowloc) and
         accumulate S.T @ msg into PSUM (segment sum in x-space); PE
         transpose of s; W matmul; ELU; write outT[:, window].
  Host:  transpose per-core outT back and concatenate.
"""

from contextlib import ExitStack
from dataclasses import dataclass

import ml_dtypes
import numpy as np

import concourse.bass as bass
import concourse.tile as tile
from concourse import bacc, library_config, mybir
from concourse.bass_utils import run_bass_kernel_spmd

F32 = mybir.dt.float32
F16 = mybir.dt.float16
BF16 = mybir.dt.bfloat16
I16 = mybir.dt.int16

P = 128
SBATCH = 8


@dataclass(frozen=True)
class Config:
    n: int = 100000          # nodes
    fin: int = 256           # input features
    fout: int = 128          # output features
    ncores: int = 8
    nchunk: int = 4          # gather-table chunks (int16 index limit)
    gwin: int = 3            # windows per gather super-group
    mbufs: int = 4           # msg tile pool depth

    @property
    def npc(self):
        return self.n // self.ncores

    @property
    def nwin(self):
        return (self.npc + P - 1) // P

    @property
    def chunk(self):
        return self.n // self.nchunk


CFG = Config()


def _ceil(a, b):
    return -(-a // b)


# --------------------------------------------------------------------------
# Static edge layout (shared across cores -> one SPMD program)
# --------------------------------------------------------------------------

@dataclass
class Layout:
    cfg: Config
    seg16: np.ndarray        # [nwin*nchunk] max-over-core segment counts
    seg_off: np.ndarray      # [nwin*nchunk] global slot offset of segment
    runs: dict               # (gi, ch) -> (start, real_len)
    groups: list             # list of window-index lists
    win_entries: dict        # w -> [(ch, blk_in_run, map_col)]
    entry_w: np.ndarray      # [n_map] window of entry
    entry_slot0: np.ndarray  # [n_map] global slot of entry's block start
    bmax: list               # per-chunk max blocks per run
    total_slots: int
    n_map: int


def build_layout(cfg: Config, row, col):
    npc, nwin, nchunk, chunk = cfg.npc, cfg.nwin, cfg.nchunk, cfg.chunk
    ngrp = nwin * nchunk

    per_core = []
    counts = np.zeros((cfg.ncores, ngrp), np.int64)
    for k in range(cfg.ncores):
        sel = (row >= k * npc) & (row < (k + 1) * npc)
        r = (row[sel] - k * npc).astype(np.int64)
        c_ = col[sel].astype(np.int64)
        w = r // P
        rl = r - w * P
        ch = c_ // chunk
        cl = c_ - ch * chunk
        key = w * nchunk + ch
        counts[k] = np.bincount(key, minlength=ngrp)
        per_core.append((key, cl, rl))

    groups = [list(range(g, min(g + cfg.gwin, nwin)))
              for g in range(0, nwin, cfg.gwin)]

    # Per-run packing: each core packs its run edges (ordered by window)
    # consecutively from the run start, so all padding is TRAILING per run
    # and can be carried as -1 indices (the Q7 desc-gen trims them for
    # free).  Window boundaries then shift per core; each window's static
    # block range covers the min/max boundary positions over all cores.
    runs = {}
    win_bounds = {}
    cur = 0
    for gi, ws in enumerate(groups):
        for ch in range(nchunk):
            start = cur
            pref = np.zeros(cfg.ncores, np.int64)
            for w in ws:
                c_k = counts[:, w * nchunk + ch]
                lo = int(pref.min())
                pref = pref + c_k
                win_bounds[(w, ch)] = (lo, int(pref.max()))
            rmax = int(pref.max())
            runs[(gi, ch)] = (start, rmax)
            cur = start + _ceil(rmax, P) * P
    total = cur

    win_entries = {}
    entry_w, entry_slot0 = [], []
    mcol = 0
    for gi, ws in enumerate(groups):
        for w in ws:
            ents = []
            for ch in range(nchunk):
                lo, hi = win_bounds[(w, ch)]
                if hi == lo:
                    continue
                rs, _ = runs[(gi, ch)]
                for b in range(lo // P, _ceil(hi, P)):
                    ents.append((ch, b, mcol))
                    entry_w.append(w)
                    entry_slot0.append(rs + b * P)
                    mcol += 1
            win_entries[w] = ents

    bmax = [max(_ceil(runs[(gi, ch)][1], P) for gi in range(len(groups)))
            for ch in range(nchunk)]
    return Layout(cfg, counts, np.zeros(1), runs, groups, win_entries,
                  np.array(entry_w), np.array(entry_slot0), bmax,
                  int(total), mcol), per_core


def build_streams(layout: Layout, key, cl, rl):
    """Per-core edge streams: wrapped int16 gather indices + per-map-entry
    row-local values."""
    cfg = layout.cfg
    total = layout.total_slots
    ngrp = cfg.nwin * cfg.nchunk

    order = np.argsort(key, kind="stable")
    skey = key[order]
    scl = cl[order]
    srl = rl[order]
    cnt = np.bincount(key, minlength=ngrp)
    starts = np.concatenate([[0], np.cumsum(cnt)[:-1]])
    rank = np.arange(len(skey)) - starts[skey]
    # per-core segment offsets: run start + this core's prefix within run
    seg_off_k = np.zeros(ngrp, np.int64)
    for gi, ws in enumerate(layout.groups):
        for ch in range(cfg.nchunk):
            rs, _ = layout.runs[(gi, ch)]
            acc = 0
            for w in ws:
                g = w * cfg.nchunk + ch
                seg_off_k[g] = rs + acc
                acc += int(cnt[g])
    slot = seg_off_k[skey] + rank

    idx_local = np.zeros(total, np.int16)
    slot_w = np.full(total, -1, np.int32)
    slot_rl = np.full(total, -1, np.int32)
    idx_local[slot] = scl.astype(np.int16)
    slot_w[slot] = skey // layout.cfg.nchunk
    slot_rl[slot] = srl

    idx_w = np.ascontiguousarray(
        np.tile(idx_local.reshape(-1, 16).T, (8, 1)))          # [128, total/16]

    pos = layout.entry_slot0[:, None] + np.arange(P)[None, :]   # [n_map, 128]
    rl_mat = np.where(slot_w[pos] == layout.entry_w[:, None],
                      slot_rl[pos], -1).astype(np.float16)
    rl_w = np.ascontiguousarray(rl_mat.T)                       # [128, n_map]
    return idx_w, rl_w


# --------------------------------------------------------------------------
# Single launch: gather x + segment-sum (one-hot matmul) + W + ELU
# --------------------------------------------------------------------------

def build_kernel(cfg: Config, layout: Layout):
    nc = bacc.Bacc("TRN2", target_bir_lowering=False, debug=False,
                   num_devices=cfg.ncores, num_swdge_queues=4)
    fin, fout, npc, nchunk, chunk = (cfg.fin, cfg.fout, cfg.npc, cfg.nchunk,
                                     cfg.chunk)
    xt = nc.dram_tensor("xt", [cfg.n, fin], F16, kind="ExternalInput")
    wt = nc.dram_tensor("wt", [fin, fout], BF16, kind="ExternalInput")
    ident = nc.dram_tensor("ident", [P, P], BF16, kind="ExternalInput")
    iota_in = nc.dram_tensor("iota", [P, SBATCH * P], F16, kind="ExternalInput")
    idxs = nc.dram_tensor("idxs", [P, layout.total_slots // 16], I16,
                          kind="ExternalInput")
    rowloc = nc.dram_tensor("rowloc", [P, layout.n_map], F16,
                            kind="ExternalInput")
    outT = nc.dram_tensor("outT", [P, npc], F32, kind="ExternalOutput")

    with tile.TileContext(nc) as tc, ExitStack() as ctx:
        nc.gpsimd.load_library(library_config.mlp)

        cpool = ctx.enter_context(tc.tile_pool(name="const", bufs=1))
        mpool = ctx.enter_context(tc.tile_pool(name="msg", bufs=cfg.mbufs))
        spool = ctx.enter_context(tc.tile_pool(name="sel", bufs=6))
        pspool = ctx.enter_context(tc.tile_pool(name="pss", bufs=2,
                                                space="PSUM"))
        ptpool = ctx.enter_context(tc.tile_pool(name="pst", bufs=2,
                                                space="PSUM"))
        popool = ctx.enter_context(tc.tile_pool(name="pso", bufs=2,
                                                space="PSUM"))
        epool = ctx.enter_context(tc.tile_pool(name="elu", bufs=3))

        # split the idx-stream load (first piece first in program order) so
        # the first group's gathers don't wait for the full stream to land
        n_groups = len(layout.groups)
        split_slot = (layout.runs[(min(1, n_groups - 1), 0)][0]
                      if n_groups > 1 else layout.total_slots)
        idx_t0 = cpool.tile([P, max(split_slot, 16) // 16], I16)
        nc.sync.dma_start(idx_t0[:], idxs.ap()[:, :split_slot // 16])
        idx_t1 = cpool.tile([P, (layout.total_slots - split_slot) // 16], I16)
        nc.sync.dma_start(idx_t1[:], idxs.ap()[:, split_slot // 16:])

        def idx_slice(lo, hi):
            if hi <= split_slot:
                return idx_t0[:, lo // 16:hi // 16]
            return idx_t1[:, (lo - split_slot) // 16:(hi - split_slot) // 16]

        iota_t = cpool.tile([P, SBATCH * P], F16)
        nc.sync.dma_start(iota_t[:], iota_in.ap()[:, :])
        rl_t = cpool.tile([P, layout.n_map], F16)
        nc.sync.dma_start(rl_t[:], rowloc.ap()[:, :])
        id_t = cpool.tile([P, P], BF16)
        nc.sync.dma_start(id_t[:], ident.ap()[:, :])
        w0_t = cpool.tile([P, fout], BF16)
        nc.sync.dma_start(w0_t[:], wt.ap()[0:P, :])
        w1_t = cpool.tile([P, fout], BF16)
        nc.sync.dma_start(w1_t[:], wt.ap()[P:2 * P, :])

        # One-time memset of every msg pool buffer: slots past a core's real
        # edge count are never DMA-written (their -1 indices are trimmed by
        # the Q7), and the 0-weighted matmul lanes need finite values.
        for _r in range(cfg.mbufs):
            for ch in range(nchunk):
                zt = mpool.tile([P, layout.bmax[ch], fin], F16,
                                name=f"mtz{ch}", tag=f"msg{ch}")
                nc.vector.memset(zt[:], 0.0)

        # SWDGE queue q is serviced by Q7 core pair (2q, 2q+1); balance the
        # descriptor-generation load by virtual finish time (uniform cost:
        # all four pairs measure ~9.27us per 1024-idx gather in isolation).
        qcost = [1.0, 1.0, 1.0, 1.0]
        vtime = [0.0, 0.0, 0.0, 0.0]
        for gi, ws in enumerate(layout.groups):
            mts = {}
            subs = []
            for ch in range(nchunk):
                start, real = layout.runs[(gi, ch)]
                if real == 0:
                    continue
                mts[ch] = mpool.tile([P, layout.bmax[ch], fin], F16,
                                     name=f"mt{ch}", tag=f"msg{ch}")
                # span ends at the 16-aligned max-over-core real count:
                # block-tail slots beyond it are never gathered (the one-time
                # pool memset keeps them finite), saving Q7 desc-gen work.
                r16 = _ceil(real, 16) * 16
                for a in range(0, r16, 1024):
                    subs.append((ch, start, a, min(a + 1024, r16)))
            subs.sort(key=lambda t: (t[2], t[0]))
            for ch, start, a, b in subs:
                q = min(range(4), key=lambda i: vtime[i])
                vtime[q] += qcost[q] * (b - a) / 1024.0
                nc.gpsimd.dma_gather(
                    mts[ch][:, a // P:_ceil(b, P), :],
                    xt.ap()[ch * chunk:(ch + 1) * chunk, :],
                    idx_slice(start + a, start + b),
                    b - a, b - a, fin,
                    single_packet=True, queue_num=q)

            for w in ws:
                entries = layout.win_entries[w]
                nt = min(npc - w * P, P)
                if not entries:
                    ot = epool.tile([P, P], F32, tag="out")
                    nc.vector.memset(ot[:], 0.0)
                    nc.sync.dma_start(outT.ap()[:, w * P:w * P + nt],
                                      ot[:, :nt])
                    continue
                ps_s = pspool.tile([P, fin], F32)
                nmm = len(entries)
                mi = 0
                bi = 0
                while bi < nmm:
                    nb = min(SBATCH, nmm - bi)
                    sel = spool.tile([P, SBATCH * P], F16, tag="sel")
                    mc0 = entries[bi][2]
                    nc.vector.tensor_tensor(
                        sel[:, :nb * P], iota_t[:, :nb * P],
                        rl_t[:, mc0:mc0 + nb].to_broadcast([P, nb, P]),
                        op=mybir.AluOpType.is_equal)
                    for j in range(nb):
                        ch, blk, _ = entries[bi + j]
                        nc.tensor.matmul(ps_s[:], sel[:, j * P:(j + 1) * P],
                                         mts[ch][:, blk, :],
                                         start=(mi == 0), stop=(mi == nmm - 1))
                        mi += 1
                    bi += nb
                # s [rows, 256] -> bf16 -> PE transpose -> sT [256-part, rows]
                # (PSUM evacuations on the Scalar engine; DVE is busy with
                # one-hot builds)
                s_sb = epool.tile([P, fin], BF16, tag="s")
                nc.scalar.activation(s_sb[:], ps_s[:],
                                     mybir.ActivationFunctionType.Copy)
                psT = ptpool.tile([P, fin], BF16)
                nc.tensor.transpose(psT[:, 0:P], s_sb[:, 0:P], id_t[:])
                nc.tensor.transpose(psT[:, P:2 * P], s_sb[:, P:2 * P], id_t[:])
                sT = epool.tile([P, fin], BF16, tag="sT")
                nc.scalar.activation(sT[:], psT[:],
                                     mybir.ActivationFunctionType.Copy)
                ps_o = popool.tile([P, fout], F32)
                nc.tensor.matmul(ps_o[:], w0_t[:], sT[:, 0:P],
                                 start=True, stop=False)
                nc.tensor.matmul(ps_o[:], w1_t[:], sT[:, P:2 * P],
                                 start=False, stop=True)
                # ELU: relu(x) - 1 + exp(min(x, 0))
                tmin = epool.tile([P, fout], F32, tag="tmin")
                texp = epool.tile([P, fout], F32, tag="texp")
                trel = epool.tile([P, fout], F32, tag="trel")
                ot = epool.tile([P, fout], F32, tag="out")
                nc.scalar.activation(tmin[:], ps_o[:],
                                     mybir.ActivationFunctionType.Relu,
                                     scale=-1.0)
                nc.scalar.activation(texp[:], tmin[:],
                                     mybir.ActivationFunctionType.Exp,
                                     scale=-1.0)
                nc.vector.tensor_scalar(trel[:], ps_o[:], 0.0, -1.0,
                                        mybir.AluOpType.max,
                                        mybir.AluOpType.add)
                nc.vector.tensor_add(ot[:], texp[:], trel[:])
                nc.sync.dma_start(outT.ap()[:, w * P:w * P + nt], ot[:, :nt])
    nc.compile()
    return nc


# --------------------------------------------------------------------------
# Host orchestration
# --------------------------------------------------------------------------

_NC_CACHE = {}


def _kernel_nc(cfg: Config, layout: Layout):
    key = (cfg.n, cfg.fin, cfg.fout, cfg.ncores, cfg.gwin, cfg.mbufs,
           tuple(sorted((k, v) for k, v in layout.runs.items())),
           tuple(layout.entry_w.tolist()))
    if key not in _NC_CACHE:
        _NC_CACHE[key] = build_kernel(cfg, layout)
    return _NC_CACHE[key]


def run(x, edge_index, W, a=None, cfg: Config = CFG, trace=False):
    """Full pipeline; returns (out, info dict with exec times)."""
    x = np.asarray(x, np.float32)
    W = np.asarray(W, np.float32)
    edge_index = np.asarray(edge_index)
    row = edge_index[0].astype(np.int64)
    col = edge_index[1].astype(np.int64)
    npc = cfg.npc
    info = {}

    layout, per_core = build_layout(cfg, row, col)
    nc = _kernel_nc(cfg, layout)

    x16 = x.astype(np.float16)
    wt = W.astype(ml_dtypes.bfloat16)
    ident = np.eye(P, dtype=ml_dtypes.bfloat16)
    iota = np.ascontiguousarray(
        np.broadcast_to(np.tile(np.arange(P, dtype=np.float16), SBATCH),
                        (P, SBATCH * P)))
    ins = []
    for k in range(cfg.ncores):
        idx_w, rl_w = build_streams(layout, *per_core[k])
        ins.append({"xt": x16, "wt": wt, "ident": ident, "iota": iota,
                    "idxs": idx_w, "rowloc": rl_w})
    r = run_bass_kernel_spmd(nc, ins, list(range(cfg.ncores)), trace=trace)
    out = np.concatenate(
        [np.ascontiguousarray(r.results[k]["outT"][:, :npc].T)
         for k in range(cfg.ncores)], axis=0)
    info["p1_ns"] = 0
    info["p2_ns"] = r.exec_time_ns
    info["total_slots"] = layout.total_slots
    info["results"] = (r,)
    return out, info


def kernel(x, edge_index, W, a=None, **_ignored):
    out, _ = run(x, edge_index, W, a)
    return out


# revision 29
# speedup vs baseline: 1.0104x; 1.0041x over previous
"""Trainium2 Bass kernel for a GAT-style GNN layer (8 NeuronCores, SPMD).

Reference computation:
    h = x @ W                                  # [N, FOUT]
    e = leakyrelu(Wh1[row] + Wh2[col])         # per-edge scores
    att = softmax(e, axis=1)                   # axis of size 1 -> exactly 1.0
    out = elu(segment_sum(att * h[col], row))  # [N, FOUT]

Because the softmax is over a size-1 axis, att == 1.0 exactly, so
out = elu(segment_sum(h[col], row)) and `a` is unused.  Since matmul
commutes with the (linear) segment sum:

    out = elu(segment_sum(x[col], row) @ W)

which lets one SPMD launch do everything: gather raw x rows per edge,
one-hot-matmul segment-sum into s = [rows, 256], transpose s on the PE,
apply W on-chip, ELU, write the (transposed) output slice.

Strategy (single SPMD launch, sharded by destination-node range):
  Host:  bucket edges by (dest window of 128 nodes, source chunk of 25000
         nodes); within each (window-group, chunk) run, segments are padded
         to the max count over cores with index-0 fills and runs are
         128-padded; blocks that straddle two windows get one one-hot
         column per window.
  Device: dma_gather x rows (512B each) from the fp16 x table on 4 SWDGE
         queues; per 128-slot block build S = (iota == rowloc) and
         accumulate S.T @ msg into PSUM (segment sum in x-space); PE
         transpose of s; W matmul; ELU; write outT[:, window].
  Host:  transpose per-core outT back and concatenate.
"""

from contextlib import ExitStack
from dataclasses import dataclass

import ml_dtypes
import numpy as np

import concourse.bass as bass
import concourse.tile as tile
from concourse import bacc, library_config, mybir
from concourse.bass_utils import run_bass_kernel_spmd

F32 = mybir.dt.float32
F16 = mybir.dt.float16
BF16 = mybir.dt.bfloat16
I16 = mybir.dt.int16

P = 128
SBATCH = 8


@dataclass(frozen=True)
class Config:
    n: int = 100000          # nodes
    fin: int = 256           # input features
    fout: int = 128          # output features
    ncores: int = 8
    nchunk: int = 4          # gather-table chunks (int16 index limit)
    gwin: int = 3            # windows per gather super-group
    mbufs: int = 4           # msg tile pool depth

    @property
    def npc(self):
        return self.n // self.ncores

    @property
    def nwin(self):
        return (self.npc + P - 1) // P

    @property
    def chunk(self):
        return self.n // self.nchunk


CFG = Config()


def _ceil(a, b):
    return -(-a // b)


# --------------------------------------------------------------------------
# Static edge layout (shared across cores -> one SPMD program)
# --------------------------------------------------------------------------

@dataclass
class Layout:
    cfg: Config
    seg16: np.ndarray        # [nwin*nchunk] max-over-core segment counts
    seg_off: np.ndarray      # [nwin*nchunk] global slot offset of segment
    runs: dict               # (gi, ch) -> (start, real_len)
    groups: list             # list of window-index lists
    win_entries: dict        # w -> [(ch, blk_in_run, map_col)]
    entry_w: np.ndarray      # [n_map] window of entry
    entry_slot0: np.ndarray  # [n_map] global slot of entry's block start
    bmax: list               # per-chunk max blocks per run
    total_slots: int
    n_map: int


def build_layout(cfg: Config, row, col):
    npc, nwin, nchunk, chunk = cfg.npc, cfg.nwin, cfg.nchunk, cfg.chunk
    ngrp = nwin * nchunk

    per_core = []
    counts = np.zeros((cfg.ncores, ngrp), np.int64)
    for k in range(cfg.ncores):
        sel = (row >= k * npc) & (row < (k + 1) * npc)
        r = (row[sel] - k * npc).astype(np.int64)
        c_ = col[sel].astype(np.int64)
        w = r // P
        rl = r - w * P
        ch = c_ // chunk
        cl = c_ - ch * chunk
        key = w * nchunk + ch
        counts[k] = np.bincount(key, minlength=ngrp)
        per_core.append((key, cl, rl))

    # exact max-over-core segment sizes: only run starts (128) and gather
    # spans (16 via the 1024-aligned subgather grid) need alignment
    seg16 = counts.max(axis=0)
    groups = [list(range(g, min(g + cfg.gwin, nwin)))
              for g in range(0, nwin, cfg.gwin)]

    seg_off = np.zeros(ngrp, np.int64)
    runs = {}
    cur = 0
    for gi, ws in enumerate(groups):
        for ch in range(nchunk):
            start = cur
            for w in ws:
                seg_off[w * nchunk + ch] = cur
                cur += int(seg16[w * nchunk + ch])
            real = cur - start
            cur = start + _ceil(real, P) * P
            runs[(gi, ch)] = (start, real)
    total = cur

    win_entries = {}
    entry_w, entry_slot0 = [], []
    mcol = 0
    for gi, ws in enumerate(groups):
        for w in ws:
            ents = []
            for ch in range(nchunk):
                g = w * nchunk + ch
                slen = int(seg16[g])
                if slen == 0:
                    continue
                rs, _ = runs[(gi, ch)]
                soff = int(seg_off[g])
                b0 = (soff - rs) // P
                b1 = _ceil(soff - rs + slen, P)
                for b in range(b0, b1):
                    ents.append((ch, b, mcol))
                    entry_w.append(w)
                    entry_slot0.append(rs + b * P)
                    mcol += 1
            win_entries[w] = ents

    bmax = [max(_ceil(runs[(gi, ch)][1], P) for gi in range(len(groups)))
            for ch in range(nchunk)]
    return Layout(cfg, seg16, seg_off, runs, groups, win_entries,
                  np.array(entry_w), np.array(entry_slot0), bmax,
                  int(total), mcol), per_core


def build_streams(layout: Layout, key, cl, rl):
    """Per-core edge streams: wrapped int16 gather indices + per-map-entry
    row-local values."""
    total = layout.total_slots
    ngrp = len(layout.seg16)

    order = np.argsort(key, kind="stable")
    skey = key[order]
    scl = cl[order]
    srl = rl[order]
    cnt = np.bincount(key, minlength=ngrp)
    starts = np.concatenate([[0], np.cumsum(cnt)[:-1]])
    rank = np.arange(len(skey)) - starts[skey]
    slot = layout.seg_off[skey] + rank

    idx_local = np.zeros(total, np.int16)
    slot_w = np.full(total, -1, np.int32)
    slot_rl = np.full(total, -1, np.int32)
    idx_local[slot] = scl.astype(np.int16)
    slot_w[slot] = skey // layout.cfg.nchunk
    slot_rl[slot] = srl

    idx_w = np.ascontiguousarray(
        np.tile(idx_local.reshape(-1, 16).T, (8, 1)))          # [128, total/16]

    pos = layout.entry_slot0[:, None] + np.arange(P)[None, :]   # [n_map, 128]
    rl_mat = np.where(slot_w[pos] == layout.entry_w[:, None],
                      slot_rl[pos], -1).astype(np.float16)
    rl_w = np.ascontiguousarray(rl_mat.T)                       # [128, n_map]
    return idx_w, rl_w


# --------------------------------------------------------------------------
# Single launch: gather x + segment-sum (one-hot matmul) + W + ELU
# --------------------------------------------------------------------------

def build_kernel(cfg: Config, layout: Layout):
    nc = bacc.Bacc("TRN2", target_bir_lowering=False, debug=False,
                   num_devices=cfg.ncores, num_swdge_queues=4)
    fin, fout, npc, nchunk, chunk = (cfg.fin, cfg.fout, cfg.npc, cfg.nchunk,
                                     cfg.chunk)
    xt = nc.dram_tensor("xt", [cfg.n, fin], F16, kind="ExternalInput")
    wt = nc.dram_tensor("wt", [fin, fout], BF16, kind="ExternalInput")
    ident = nc.dram_tensor("ident", [P, P], BF16, kind="ExternalInput")
    iota_in = nc.dram_tensor("iota", [P, SBATCH * P], F16, kind="ExternalInput")
    idxs = nc.dram_tensor("idxs", [P, layout.total_slots // 16], I16,
                          kind="ExternalInput")
    rowloc = nc.dram_tensor("rowloc", [P, layout.n_map], F16,
                            kind="ExternalInput")
    outT = nc.dram_tensor("outT", [P, npc], F32, kind="ExternalOutput")

    with tile.TileContext(nc) as tc, ExitStack() as ctx:
        nc.gpsimd.load_library(library_config.mlp)

        cpool = ctx.enter_context(tc.tile_pool(name="const", bufs=1))
        mpool = ctx.enter_context(tc.tile_pool(name="msg", bufs=cfg.mbufs))
        spool = ctx.enter_context(tc.tile_pool(name="sel", bufs=6))
        pspool = ctx.enter_context(tc.tile_pool(name="pss", bufs=2,
                                                space="PSUM"))
        ptpool = ctx.enter_context(tc.tile_pool(name="pst", bufs=2,
                                                space="PSUM"))
        popool = ctx.enter_context(tc.tile_pool(name="pso", bufs=2,
                                                space="PSUM"))
        epool = ctx.enter_context(tc.tile_pool(name="elu", bufs=3))

        # split the idx-stream load (first piece first in program order) so
        # the first group's gathers don't wait for the full stream to land
        n_groups = len(layout.groups)
        split_slot = (layout.runs[(min(1, n_groups - 1), 0)][0]
                      if n_groups > 1 else layout.total_slots)
        idx_t0 = cpool.tile([P, max(split_slot, 16) // 16], I16)
        nc.sync.dma_start(idx_t0[:], idxs.ap()[:, :split_slot // 16])
        idx_t1 = cpool.tile([P, (layout.total_slots - split_slot) // 16], I16)
        nc.sync.dma_start(idx_t1[:], idxs.ap()[:, split_slot // 16:])

        def idx_slice(lo, hi):
            if hi <= split_slot:
                return idx_t0[:, lo // 16:hi // 16]
            return idx_t1[:, (lo - split_slot) // 16:(hi - split_slot) // 16]

        iota_t = cpool.tile([P, SBATCH * P], F16)
        nc.sync.dma_start(iota_t[:], iota_in.ap()[:, :])
        rl_t = cpool.tile([P, layout.n_map], F16)
        nc.sync.dma_start(rl_t[:], rowloc.ap()[:, :])
        id_t = cpool.tile([P, P], BF16)
        nc.sync.dma_start(id_t[:], ident.ap()[:, :])
        w0_t = cpool.tile([P, fout], BF16)
        nc.sync.dma_start(w0_t[:], wt.ap()[0:P, :])
        w1_t = cpool.tile([P, fout], BF16)
        nc.sync.dma_start(w1_t[:], wt.ap()[P:2 * P, :])

        # SWDGE queue q is serviced by Q7 core pair (2q, 2q+1); balance the
        # descriptor-generation load by virtual finish time (uniform cost:
        # all four pairs measure ~9.27us per 1024-idx gather in isolation).
        qcost = [1.0, 1.0, 1.0, 1.0]
        vtime = [0.0, 0.0, 0.0, 0.0]
        for gi, ws in enumerate(layout.groups):
            mts = {}
            subs = []
            for ch in range(nchunk):
                start, real = layout.runs[(gi, ch)]
                if real == 0:
                    continue
                mts[ch] = mpool.tile([P, layout.bmax[ch], fin], F16,
                                     name=f"mt{ch}", tag=f"msg{ch}")
                # gather the full 128-padded span: pad slots carry index 0,
                # so every consumed slot is DMA-written (no NaN garbage).
                r128 = _ceil(real, P) * P
                for a in range(0, r128, 1024):
                    subs.append((ch, start, a, min(a + 1024, r128)))
            subs.sort(key=lambda t: (t[2], t[0]))
            for ch, start, a, b in subs:
                q = min(range(4), key=lambda i: vtime[i])
                vtime[q] += qcost[q] * (b - a) / 1024.0
                nc.gpsimd.dma_gather(
                    mts[ch][:, a // P:b // P, :],
                    xt.ap()[ch * chunk:(ch + 1) * chunk, :],
                    idx_slice(start + a, start + b),
                    b - a, b - a, fin,
                    single_packet=True, queue_num=q)

            for w in ws:
                entries = layout.win_entries[w]
                nt = min(npc - w * P, P)
                if not entries:
                    ot = epool.tile([P, P], F32, tag="out")
                    nc.vector.memset(ot[:], 0.0)
                    nc.sync.dma_start(outT.ap()[:, w * P:w * P + nt],
                                      ot[:, :nt])
                    continue
                ps_s = pspool.tile([P, fin], F32)
                nmm = len(entries)
                mi = 0
                bi = 0
                while bi < nmm:
                    nb = min(SBATCH, nmm - bi)
                    sel = spool.tile([P, SBATCH * P], F16, tag="sel")
                    mc0 = entries[bi][2]
                    nc.vector.tensor_tensor(
                        sel[:, :nb * P], iota_t[:, :nb * P],
                        rl_t[:, mc0:mc0 + nb].to_broadcast([P, nb, P]),
                        op=mybir.AluOpType.is_equal)
                    for j in range(nb):
                        ch, blk, _ = entries[bi + j]
                        nc.tensor.matmul(ps_s[:], sel[:, j * P:(j + 1) * P],
                                         mts[ch][:, blk, :],
                                         start=(mi == 0), stop=(mi == nmm - 1))
                        mi += 1
                    bi += nb
                # s [rows, 256] -> bf16 -> PE transpose -> sT [256-part, rows]
                # (PSUM evacuations on the Scalar engine; DVE is busy with
                # one-hot builds)
                s_sb = epool.tile([P, fin], BF16, tag="s")
                nc.scalar.activation(s_sb[:], ps_s[:],
                                     mybir.ActivationFunctionType.Copy)
                psT = ptpool.tile([P, fin], BF16)
                nc.tensor.transpose(psT[:, 0:P], s_sb[:, 0:P], id_t[:])
                nc.tensor.transpose(psT[:, P:2 * P], s_sb[:, P:2 * P], id_t[:])
                sT = epool.tile([P, fin], BF16, tag="sT")
                nc.scalar.activation(sT[:], psT[:],
                                     mybir.ActivationFunctionType.Copy)
                ps_o = popool.tile([P, fout], F32)
                nc.tensor.matmul(ps_o[:], w0_t[:], sT[:, 0:P],
                                 start=True, stop=False)
                nc.tensor.matmul(ps_o[:], w1_t[:], sT[:, P:2 * P],
                                 start=False, stop=True)
                # ELU: relu(x) - 1 + exp(min(x, 0))
                tmin = epool.tile([P, fout], F32, tag="tmin")
                texp = epool.tile([P, fout], F32, tag="texp")
                trel = epool.tile([P, fout], F32, tag="trel")
                ot = epool.tile([P, fout], F32, tag="out")
                nc.scalar.activation(tmin[:], ps_o[:],
                                     mybir.ActivationFunctionType.Relu,
                                     scale=-1.0)
                nc.scalar.activation(texp[:], tmin[:],
                                     mybir.ActivationFunctionType.Exp,
                                     scale=-1.0)
                nc.vector.tensor_scalar(trel[:], ps_o[:], 0.0, -1.0,
                                        mybir.AluOpType.max,
                                        mybir.AluOpType.add)
                nc.vector.tensor_add(ot[:], texp[:], trel[:])
                nc.sync.dma_start(outT.ap()[:, w * P:w * P + nt], ot[:, :nt])
    nc.compile()
    return nc


# --------------------------------------------------------------------------
# Host orchestration
# --------------------------------------------------------------------------

_NC_CACHE = {}


def _kernel_nc(cfg: Config, layout: Layout):
    key = (cfg.n, cfg.fin, cfg.fout, cfg.ncores, cfg.gwin, cfg.mbufs,
           tuple(layout.seg16.tolist()))
    if key not in _NC_CACHE:
        _NC_CACHE[key] = build_kernel(cfg, layout)
    return _NC_CACHE[key]


def run(x, edge_index, W, a=None, cfg: Config = CFG, trace=False):
    """Full pipeline; returns (out, info dict with exec times)."""
    x = np.asarray(x, np.float32)
    W = np.asarray(W, np.float32)
    edge_index = np.asarray(edge_index)
    row = edge_index[0].astype(np.int64)
    col = edge_index[1].astype(np.int64)
    npc = cfg.npc
    info = {}

    layout, per_core = build_layout(cfg, row, col)
    nc = _kernel_nc(cfg, layout)

    x16 = x.astype(np.float16)
    wt = W.astype(ml_dtypes.bfloat16)
    ident = np.eye(P, dtype=ml_dtypes.bfloat16)
    iota = np.ascontiguousarray(
        np.broadcast_to(np.tile(np.arange(P, dtype=np.float16), SBATCH),
                        (P, SBATCH * P)))
    ins = []
    for k in range(cfg.ncores):
        idx_w, rl_w = build_streams(layout, *per_core[k])
        ins.append({"xt": x16, "wt": wt, "ident": ident, "iota": iota,
                    "idxs": idx_w, "rowloc": rl_w})
    r = run_bass_kernel_spmd(nc, ins, list(range(cfg.ncores)), trace=trace)
    out = np.concatenate(
        [np.ascontiguousarray(r.results[k]["outT"][:, :npc].T)
         for k in range(cfg.ncores)], axis=0)
    info["p1_ns"] = 0
    info["p2_ns"] = r.exec_time_ns
    info["total_slots"] = layout.total_slots
    info["results"] = (r,)
    return out, info


def kernel(x, edge_index, W, a=None, **_ignored):
    out, _ = run(x, edge_index, W, a)
    return out
